# revision 1
# baseline (speedup 1.0000x reference)
"""BasicRGCN Trainium2 kernel (8 NeuronCores, SPMD).

Math (reference):
    x = features                                   # [N, F]
    for l in 0..1:
        y = sum_r A[r] @ x @ W[l, r].T             # [N, F]
        x = sigmoid(y)
    out[r] = (x @ M_r) @ x.T                       # [R, N, N]

Sharding: node rows N split across 8 cores (512 rows each). Each core holds
its adjacency row-slab (pre-transposed on host to [m, n_local] tile layout so
the contraction dim m lands on SBUF partitions) and computes its slab of the
output. The tiny [N, F] activations are all-gathered between layers.

Precision strategy:
  * Layer matmuls run with fp8e4m3 adjacency + fp8 per-relation projected
    activations (h_r = x @ W_r.T), accumulating fp32 in PSUM. Host-side
    simulation shows this is exact for the final output in this regime (the
    layer-2 pre-activations are ~5e4, so sigmoid saturates hard).
  * The adjacency slab (8 MiB/core in fp8) stays resident in SBUF across both
    layers, so HBM reads it once.
  * The DistMult phase needs real fp32 accuracy: operands are split into
    fp16 hi + fp16 lo (x = hi + lo, |lo| <~ 2^-11 |x|), and each output tile
    accumulates hi*hi + hi*lo + lo*hi on PSUM. Error ~2^-22, at full fp16
    matmul throughput. The tiny xm = x2 @ M_r matmul runs in true fp32.

Performance notes (all empirically measured on this runtime):
  * A single dma_start runs on one DMA engine (~30 GB/s), and each queue
    family (HWDGE via nc.sync, SWDGE via nc.gpsimd) alone tops out near
    240 GB/s, which is also about the per-core HBM limit here (LNC1 pairs
    share an HBM port). All bulk transfers are therefore split into many
    DMAs spread over both families, with HBM-contiguous runs (the host
    pre-tiles the adjacency so DMA source runs are 4 KiB, and output
    row-blocks are staged [128, 4096] so stores are fully contiguous).
  * Both all-gathers are padded to 1 MiB gathered output so the collective
    picks RDH (~22 us) instead of Mesh (measured 49 us at 512 KiB). The
    first all-gather additionally absorbs the per-core NEFF launch skew.
  * If the PE idles >~3.4 us it is re-throttled to 1.2 GHz (HAM clock gate)
    and, measured on this kernel, a pure back-to-back matmul stream does not
    recover to 2.4 GHz. Scratch matmuls on pre-collective data keep the PE
    busy across the first all-gather so h2/layer-2 run at full clock. The
    DistMult phase needs no keep-warm: it is store-bandwidth-bound and even
    a cold PE outpaces the stores.
"""

import numpy as np
import ml_dtypes

import concourse.bacc as bacc
import concourse.mybir as mybir
import concourse.tile as tile
from concourse import bass_utils

R, N, F = 4, 4096, 64
NCORES = 8
NL = N // NCORES          # 512 local node rows per core
MB = N // 128             # 32 contraction blocks of 128
NB = NL // 128            # 4 output row-blocks per core
MC = N // 512             # 8 output column-chunks

WARM0 = 16                # pre-warm matmuls at kernel start
WARM1 = 200               # keep-warm matmuls across all-gather 1

F8NP = ml_dtypes.float8_e4m3fn
F8 = mybir.dt.float8e4
F16 = mybir.dt.float16
F32 = mybir.dt.float32

# Set by the test harness to collect a profile; grading path leaves these alone.
TRACE = False
LAST_RESULT = None

_NC_CACHE = None


def _build():
    nc = bacc.Bacc("TRN2", target_bir_lowering=False, debug=False,
                   num_devices=NCORES)

    # Per-core inputs (host pre-laid-out; see kernel() below).
    atr = nc.dram_tensor("atr", [R, 128, MB, NL], F8, kind="ExternalInput")
    h1 = nc.dram_tensor("h1", [128, R * MB * F], F8, kind="ExternalInput")
    wt2 = nc.dram_tensor("wt2", [F, R * F], F16, kind="ExternalInput")
    relm = nc.dram_tensor("relm", [F, R * F], F32, kind="ExternalInput")
    out = nc.dram_tensor("out", [R, NL, N], F32, kind="ExternalOutput")

    rg = [list(range(NCORES))]
    SIG = mybir.ActivationFunctionType.Sigmoid

    with tile.TileContext(nc) as tc:
        with (
            tc.tile_pool(name="big", bufs=1) as big,
            tc.tile_pool(name="sb", bufs=1) as sb,
            tc.tile_pool(name="stage", bufs=4) as stage,
            tc.tile_pool(name="ps", bufs=1, space="PSUM") as ps,
            tc.tile_pool(name="psh", bufs=3, space="PSUM") as psh,
            tc.tile_pool(name="pso", bufs=3, space="PSUM") as pso,
            tc.tile_pool(name="dram", bufs=1, space="DRAM") as dram,
        ):
            # Adjacency slab, resident in SBUF across both layers: fp8, 64KB/partition.
            a_res = big.tile([128, R * MB * NL], F8)
            a_v = a_res.rearrange("p (r mb j) -> p r mb j", r=R, mb=MB)

            # Layer-1 projected activations h1[p, r, mb, g], from host.
            h1_sb = sb.tile([128, R * MB * F], F8)
            HC = R * MB * F // 4
            for q in range(4):
                eng = nc.sync if q % 2 == 0 else nc.gpsimd
                eng.dma_start(h1_sb[:, q * HC:(q + 1) * HC],
                              h1[:, q * HC:(q + 1) * HC])
            h1_v = h1_sb.rearrange("p (r mb g) -> p r mb g", r=R, mb=MB)

            wt2_sb = sb.tile([F, R * F], F16)
            nc.sync.dma_start(wt2_sb[:], wt2[:])
            relm_sb = sb.tile([F, R * F], F32)
            nc.sync.dma_start(relm_sb[:], relm[:])

            # All-gather pack buffers (padded to 1 MiB gathered so the
            # collective picks RDH, not Mesh). Pad halves zeroed up front.
            x1pack = sb.tile([F, 2 * NL], F16)
            x2pack = sb.tile([F, 2 * NL], F16)
            nc.gpsimd.memset(x1pack[:, NL:], 0.0)
            scratch = ps.tile([F, NL], F32, tag="warm")

            # Adjacency loads: 16 DMAs split across HWDGE (sync) and SWDGE
            # (gpsimd) queue families - either family alone caps at ~240 GB/s.
            H = MB // 4
            for r in range(R):
                for h in range(4):
                    eng = nc.sync if (r * 4 + h) % 2 == 0 else nc.gpsimd
                    eng.dma_start(
                        a_v[:, r, h * H:(h + 1) * H, :],
                        atr[r, :, h * H:(h + 1) * H, :],
                    )

            # Pre-warm the PE while the adjacency stream lands.
            for _ in range(WARM0):
                nc.tensor.matmul(scratch[:], h1_v[:, 0, 0, :],
                                 h1_sb[:, 0:NL], start=True, stop=True)

            # ---- Layer 1: yT[g, n_local] = sum_{r, m} h1_r[m, g] * A[r, n, m]
            y1 = ps.tile([F, NL], F32, tag="y")
            k = 0
            for r in range(R):
                for mb in range(MB):
                    nc.tensor.matmul(
                        y1[:], h1_v[:, r, mb, :], a_v[:, r, mb, :],
                        start=(k == 0), stop=(k == R * MB - 1),
                    )
                    k += 1
            nc.scalar.activation(x1pack[:, 0:NL], y1[:], SIG)

            # ---- All-gather x1 (fp16, padded): [F, 2*NL] -> 8 x [F, 2*NL]
            b1_in = dram.tile([F, 2 * NL], F16)
            b1_out = dram.tile([NCORES, F, 2 * NL], F16, addr_space="Shared")
            nc.sync.dma_start(b1_in[:], x1pack[:])
            nc.gpsimd.collective_compute(
                "AllGather", mybir.AluOpType.bypass, replica_groups=rg,
                ins=[b1_in[:]], outs=[b1_out[:]],
            )
            # Keep the PE busy (HAM stays at 2.4 GHz) while the collective runs.
            for _ in range(WARM1):
                nc.tensor.matmul(scratch[:], x1pack[:, 0:F], x1pack[:, 0:NL],
                                 start=True, stop=True)
            # Load gathered x1 in 4 chunks (parallel DMA queues, earlier h2 start).
            x1t = sb.tile([F, N], F16)
            for q in range(NCORES):
                eng = nc.sync if q % 2 == 0 else nc.gpsimd
                eng.dma_start(
                    x1t[:, q * NL:(q + 1) * NL],
                    b1_out[q, :, 0:NL],
                )

            # ---- h2[m, (r, g)] = x1[m, :] @ W2r.T for all r (cast to fp8)
            h2_sb = sb.tile([128, R * MB * F], F8)
            h2_v = h2_sb.rearrange("p (r mb g) -> p r mb g", r=R, mb=MB)
            for mb in range(MB):
                ph = psh.tile([128, R * F], F32, tag="h")
                nc.tensor.matmul(ph[:], x1t[:, mb * 128:(mb + 1) * 128],
                                 wt2_sb[:], start=True, stop=True)
                nc.vector.tensor_copy(
                    h2_v[:, :, mb, :],
                    ph[:].rearrange("p (r g) -> p r g", r=R),
                )

            # ---- Layer 2 (adjacency already resident in SBUF)
            y2 = ps.tile([F, NL], F32, tag="y")
            k = 0
            for r in range(R):
                for mb in range(MB):
                    nc.tensor.matmul(
                        y2[:], h2_v[:, r, mb, :], a_v[:, r, mb, :],
                        start=(k == 0), stop=(k == R * MB - 1),
                    )
                    k += 1
            x2t_loc = sb.tile([F, NL], F32)
            nc.scalar.activation(x2t_loc[:], y2[:], SIG)

            # ---- Split local x2 into fp16 hi/lo, packed for a single gather
            nc.vector.tensor_copy(x2pack[:, 0:NL], x2t_loc[:])
            nc.vector.tensor_sub(x2pack[:, NL:2 * NL], x2t_loc[:],
                                 x2pack[:, 0:NL])

            # ---- All-gather packed x2 hi/lo: [F, 2*NL] -> [F, 2*N]
            b2_in = dram.tile([F, 2 * NL], F16)
            b2_out = dram.tile([NCORES, F, 2 * NL], F16, addr_space="Shared")
            nc.sync.dma_start(b2_in[:], x2pack[:])
            nc.gpsimd.collective_compute(
                "AllGather", mybir.AluOpType.bypass, replica_groups=rg,
                ins=[b2_in[:]], outs=[b2_out[:]],
            )
            # ---- xmT[r] = (x2_local @ M_r).T in true fp32, split hi/lo.
            # hi lands on partitions 0-63 of xm_hl, lo on 64-127 (via an
            # SBUF->SBUF DMA partition move), so the hi*hi and lo*hi terms of
            # the DistMult matmul fuse into ONE K=128 matmul against x2hh
            # (x2_hi duplicated on both partition halves).
            xm_hl = sb.tile([128, R * NL], F16)
            xm_hl_v = xm_hl.rearrange("p (r j) -> p r j", r=R)
            xmlo_tmp = sb.tile([F, R * NL], F16)
            xmlo_tmp_v = xmlo_tmp.rearrange("g (r j) -> g r j", r=R)
            for r in range(R):
                pxm = psh.tile([F, NL], F32, tag="h")
                nc.tensor.matmul(pxm[:], relm_sb[:, r * F:(r + 1) * F],
                                 x2t_loc[:], start=True, stop=True)
                nc.vector.tensor_copy(xm_hl_v[0:F, r, :], pxm[:])
                nc.vector.tensor_sub(xmlo_tmp_v[:, r, :], pxm[:],
                                     xm_hl_v[0:F, r, :])
            nc.sync.dma_start(xm_hl[F:128, :], xmlo_tmp[:])

            # Load gathered x2: hi duplicated onto both partition halves of
            # x2hh, lo separate. Chunked for parallel DMA queues.
            x2hh = sb.tile([128, N], F16)
            x2lo = sb.tile([F, N], F16)
            b2_v = b2_out.rearrange("c g (h j) -> c g h j", h=2)
            for q in range(NCORES):
                eng = nc.sync if q % 2 == 0 else nc.gpsimd
                eng.dma_start(x2hh[0:F, q * NL:(q + 1) * NL],
                              b2_v[q, :, 0, :])
                eng2 = nc.gpsimd if q % 2 == 0 else nc.sync
                eng2.dma_start(x2lo[:, q * NL:(q + 1) * NL],
                               b2_v[q, :, 1, :])
            # duplicate hi onto the upper partition half (SBUF->SBUF, off HBM)
            for q in range(2):
                half_n = slice(q * (N // 2), (q + 1) * (N // 2))
                (nc.sync if q == 0 else nc.gpsimd).dma_start(
                    x2hh[F:128, half_n], x2hh[0:F, half_n])

            # ---- DistMult scores: out[r, n, m] = sum_g xm[r][n, g] x2[m, g]
            # Two 512-col chunks share one staging tile / one store DMA.
            for r in range(R):
                for nb in range(NB):
                    lhs_hl = xm_hl_v[:, r, nb * 128:(nb + 1) * 128]
                    lhs_hi = xm_hl_v[0:F, r, nb * 128:(nb + 1) * 128]
                    so = stage.tile([128, N], F32, tag="so", bufs=3)
                    for mc in range(MC):
                        cs = slice(mc * 512, (mc + 1) * 512)
                        po = pso.tile([128, 512], F32, tag="o")
                        nc.tensor.matmul(po[:], lhs_hl, x2hh[:, cs],
                                         start=True, stop=False)
                        nc.tensor.matmul(po[:], lhs_hi, x2lo[:, cs],
                                         start=False, stop=True)
                        if mc % 2 == 0:
                            nc.vector.tensor_copy(so[:, cs], po[:])
                        else:
                            nc.scalar.copy(so[:, cs], po[:])
                    # Store the full row-block as 4 fully-contiguous 512 KiB
                    # DMAs spread over both queue families (a single dma_start
                    # runs on one engine at ~30 GB/s; HBM needs ~12 engines).
                    for ps_ in range(4):
                        seng = nc.sync if ps_ % 2 == 0 else nc.gpsimd
                        seng.dma_start(
                            out[r, nb * 128 + ps_ * 32:
                                nb * 128 + (ps_ + 1) * 32, :],
                            so[ps_ * 32:(ps_ + 1) * 32, :],
                        )
    nc.compile()
    return nc


def _get_nc():
    global _NC_CACHE
    if _NC_CACHE is None:
        _NC_CACHE = _build()
    return _NC_CACHE


def kernel(**inputs):
    global LAST_RESULT
    A = np.asarray(inputs["adjacency"], dtype=np.float32)
    x0 = np.asarray(inputs["features"], dtype=np.float32)
    W = np.asarray(inputs["conv_weights"], dtype=np.float32)
    Mrel = np.asarray(inputs["rel_matrices"], dtype=np.float32)

    # h1[r, m, g] = sum_f x0[m, f] * W[0, r, g, f]; SBUF layout [p, r, mb, g].
    h1 = np.einsum("mf,rgf->rmg", x0, W[0])
    h1_tiled = np.ascontiguousarray(
        h1.reshape(R, MB, 128, F).transpose(2, 0, 1, 3)
    ).reshape(128, R * MB * F).astype(F8NP)
    # wt2[f, (r, g)] = W[1, r, g, f]
    wt2 = np.ascontiguousarray(
        W[1].transpose(2, 0, 1)).reshape(F, R * F).astype(np.float16)
    # relm[g1, (r, g2)] = M[r, g1, g2]
    relm = np.ascontiguousarray(
        Mrel.transpose(1, 0, 2)).reshape(F, R * F).astype(np.float32)

    nc = _get_nc()
    in_maps = []
    for c in range(NCORES):
        sl = A[:, c * NL:(c + 1) * NL, :]             # [R, NL, N]
        atr = np.ascontiguousarray(
            sl.transpose(0, 2, 1)                      # [R, N(m), NL(j)]
            .reshape(R, MB, 128, NL)
            .transpose(0, 2, 1, 3)                     # [R, p, mb, j]
        ).astype(F8NP)
        in_maps.append(dict(atr=atr, h1=h1_tiled, wt2=wt2, relm=relm))

    res = bass_utils.run_bass_kernel_spmd(
        nc, in_maps, core_ids=list(range(NCORES)), trace=TRACE,
    )
    LAST_RESULT = res

    out = np.empty((R, N, N), dtype=np.float32)
    for c in range(NCORES):
        out[:, c * NL:(c + 1) * NL, :] = res.results[c]["out"]
    return out



# revision 2
# speedup vs baseline: 1.3628x; 1.3628x over previous
"""BasicRGCN Trainium2 kernel (8 NeuronCores, SPMD).

Math (reference):
    x = features                                   # [N, F]
    for l in 0..1:
        y = sum_r A[r] @ x @ W[l, r].T             # [N, F]
        x = sigmoid(y)
    out[r] = (x @ M_r) @ x.T                       # [R, N, N]

Sharding: node rows N split across 8 cores (512 rows each). Each core holds
its adjacency row-slab (pre-transposed on host to [m, n_local] tile layout so
the contraction dim m lands on SBUF partitions) and computes its slab of the
output. The tiny [N, F] activations are all-gathered between layers.

Precision strategy:
  * Layer matmuls run with fp8e4m3 adjacency + fp8 per-relation projected
    activations (h_r = x @ W_r.T), accumulating fp32 in PSUM. The layer-2
    pre-activations are ~5e4, so sigmoid saturates hard and layer precision
    is absorbed entirely.
  * The DistMult phase runs in plain fp16 (x2 and xm = x2 @ M_r both fp16,
    fp32 PSUM accumulation). Worst-case rel err ~4e-5 on this regime - far
    under the 2e-2 gate - at a single K=64 matmul per output tile.

Schedule notes (from ntff profiles of prior revisions):
  * A dummy 2KiB AllGather is triggered as the very first instruction: the
    collectives runtime pays a ~38us init/launch-skew barrier on the first
    cc op, which this overlaps with the adjacency load + layer-1 compute
    instead of serializing before the first real all-gather.
  * The adjacency slab (8 MiB/core in fp8) stays resident in SBUF across
    both layers, so HBM reads it once.
  * All-gathers are padded to 1 MiB gathered output so the collective picks
    RDH instead of Mesh.
  * Scratch matmuls keep the PE busy (HAM stays un-throttled) across the
    first all-gather. The DistMult phase needs no keep-warm: even at the
    cold-isolated matmul rate (~(219+N)/1.2GHz) it outpaces the stores.
  * Output stores are one fully-contiguous 2 MiB DMA per 128-row block,
    rotated across the three DGE rings (sync=SP-HWDGE, gpsimd=SWDGE,
    scalar=ACT-HWDGE); per-ring FIFO serialization is what capped the
    store phase previously.
"""

import numpy as np
import ml_dtypes

import concourse.bacc as bacc
import concourse.mybir as mybir
import concourse.tile as tile
from concourse import bass_utils

R, N, F = 4, 4096, 64
NCORES = 8
NL = N // NCORES          # 512 local node rows per core
MB = N // 128             # 32 contraction blocks of 128
NB = NL // 128            # 4 output row-blocks per core
MC = N // 512             # 8 output column-chunks

WARM0 = 16                # pre-warm matmuls at kernel start
WARM1 = 80                # keep-warm matmuls across all-gather 1
PAD = True                # pad gathers to 1 MiB output (RDH algo)

F8NP = ml_dtypes.float8_e4m3fn
F8 = mybir.dt.float8e4
F16 = mybir.dt.float16
F32 = mybir.dt.float32

# Set by the test harness to collect a profile; grading path leaves these alone.
TRACE = False
LAST_RESULT = None

_NC_CACHE = None


def _build():
    nc = bacc.Bacc("TRN2", target_bir_lowering=False, debug=False,
                   num_devices=NCORES)

    # Per-core inputs (host pre-laid-out; see kernel() below).
    atr = nc.dram_tensor("atr", [R, 128, MB, NL], F8, kind="ExternalInput")
    h1 = nc.dram_tensor("h1", [128, R * MB * F], F8, kind="ExternalInput")
    wt2 = nc.dram_tensor("wt2", [F, R * F], F16, kind="ExternalInput")
    relm = nc.dram_tensor("relm", [F, R * F], F16, kind="ExternalInput")
    out = nc.dram_tensor("out", [R, NL, N], F32, kind="ExternalOutput")

    rg = [list(range(NCORES))]
    SIG = mybir.ActivationFunctionType.Sigmoid
    PADN = 2 * NL if PAD else NL

    with tile.TileContext(nc) as tc:
        with (
            tc.tile_pool(name="big", bufs=1) as big,
            tc.tile_pool(name="sb", bufs=1) as sb,
            tc.tile_pool(name="stage", bufs=3) as stage,
            tc.tile_pool(name="ps", bufs=1, space="PSUM") as ps,
            tc.tile_pool(name="psh", bufs=3, space="PSUM") as psh,
            tc.tile_pool(name="pso", bufs=3, space="PSUM") as pso,
            tc.tile_pool(name="dram", bufs=1, space="DRAM") as dram,
        ):
            # ---- CC warmup: trigger a dummy tiny AllGather first thing.
            # Nothing reads its output; its only purpose is to pull the
            # collectives init / launch-skew barrier into the adjacency-load
            # window. Input DRAM is uninitialized - AllGather only copies.
            cw_in = dram.tile([F, 16], F16)
            cw_out = dram.tile([NCORES, F, 16], F16, addr_space="Shared")
            nc.gpsimd.collective_compute(
                "AllGather", mybir.AluOpType.bypass, replica_groups=rg,
                ins=[cw_in[:]], outs=[cw_out[:]],
            )

            # Adjacency slab, resident in SBUF across both layers: fp8, 64KB/partition.
            a_res = big.tile([128, R * MB * NL], F8)
            a_v = a_res.rearrange("p (r mb j) -> p r mb j", r=R, mb=MB)

            # Layer-1 projected activations h1[p, r, mb, g], from host.
            h1_sb = sb.tile([128, R * MB * F], F8)
            HC = R * MB * F // 4
            for q in range(4):
                eng = nc.sync if q % 2 == 0 else nc.gpsimd
                eng.dma_start(h1_sb[:, q * HC:(q + 1) * HC],
                              h1[:, q * HC:(q + 1) * HC])
            h1_v = h1_sb.rearrange("p (r mb g) -> p r mb g", r=R, mb=MB)

            wt2_sb = sb.tile([F, R * F], F16)
            nc.sync.dma_start(wt2_sb[:], wt2[:])
            relm_sb = sb.tile([F, R * F], F16)
            nc.sync.dma_start(relm_sb[:], relm[:])

            # All-gather pack buffers (fp16). Pad region of the DRAM staging
            # tiles is uninitialized garbage - it is gathered but never read.
            x1pack = sb.tile([F, NL], F16)
            x2pack = sb.tile([F, NL], F16)
            scratch = ps.tile([F, NL], F32, tag="warm")

            # Adjacency loads: 16 DMAs split across HWDGE (sync) and SWDGE
            # (gpsimd) queue families - either family alone caps at ~240 GB/s.
            H = MB // 4
            for r in range(R):
                for h in range(4):
                    eng = nc.sync if (r * 4 + h) % 2 == 0 else nc.gpsimd
                    eng.dma_start(
                        a_v[:, r, h * H:(h + 1) * H, :],
                        atr[r, :, h * H:(h + 1) * H, :],
                    )

            # Pre-warm the PE while the adjacency stream lands.
            for _ in range(WARM0):
                nc.tensor.matmul(scratch[:], h1_v[:, 0, 0, :],
                                 h1_sb[:, 0:NL], start=True, stop=True)

            # ---- Layer 1: yT[g, n_local] = sum_{r, m} h1_r[m, g] * A[r, n, m]
            y1 = ps.tile([F, NL], F32, tag="y")
            k = 0
            for r in range(R):
                for mb in range(MB):
                    nc.tensor.matmul(
                        y1[:], h1_v[:, r, mb, :], a_v[:, r, mb, :],
                        start=(k == 0), stop=(k == R * MB - 1),
                    )
                    k += 1
            nc.scalar.activation(x1pack[:], y1[:], SIG)

            # ---- All-gather x1 (fp16, padded): [F, PADN] -> 8 x [F, PADN]
            b1_in = dram.tile([F, PADN], F16)
            b1_out = dram.tile([NCORES, F, PADN], F16, addr_space="Shared")
            nc.sync.dma_start(b1_in[:, 0:NL], x1pack[:])
            nc.gpsimd.collective_compute(
                "AllGather", mybir.AluOpType.bypass, replica_groups=rg,
                ins=[b1_in[:]], outs=[b1_out[:]],
            )
            # Keep the PE busy (HAM stays at 2.4 GHz) while the collective runs.
            for _ in range(WARM1):
                nc.tensor.matmul(scratch[:], x1pack[:, 0:F], x1pack[:, 0:NL],
                                 start=True, stop=True)
            # Load gathered x1 in 8 chunks (parallel DMA queues, earlier h2 start).
            x1t = sb.tile([F, N], F16)
            for q in range(NCORES):
                eng = nc.sync if q % 2 == 0 else nc.gpsimd
                eng.dma_start(
                    x1t[:, q * NL:(q + 1) * NL],
                    b1_out[q, :, 0:NL],
                )

            # ---- h2[m, (r, g)] = x1[m, :] @ W2r.T for all r (cast to fp8)
            h2_sb = sb.tile([128, R * MB * F], F8)
            h2_v = h2_sb.rearrange("p (r mb g) -> p r mb g", r=R, mb=MB)
            for mb in range(MB):
                ph = psh.tile([128, R * F], F32, tag="h")
                nc.tensor.matmul(ph[:], x1t[:, mb * 128:(mb + 1) * 128],
                                 wt2_sb[:], start=True, stop=True)
                ceng = nc.vector.tensor_copy if mb % 2 == 0 else nc.scalar.copy
                ceng(
                    h2_v[:, :, mb, :],
                    ph[:].rearrange("p (r g) -> p r g", r=R),
                )

            # ---- Layer 2 (adjacency already resident in SBUF)
            y2 = ps.tile([F, NL], F32, tag="y")
            k = 0
            for r in range(R):
                for mb in range(MB):
                    nc.tensor.matmul(
                        y2[:], h2_v[:, r, mb, :], a_v[:, r, mb, :],
                        start=(k == 0), stop=(k == R * MB - 1),
                    )
                    k += 1
            # sigmoid straight to fp16 pack buffer
            nc.scalar.activation(x2pack[:], y2[:], SIG)

            # ---- All-gather x2 (fp16, padded): [F, PADN] -> 8 x [F, PADN]
            b2_in = dram.tile([F, PADN], F16)
            b2_out = dram.tile([NCORES, F, PADN], F16, addr_space="Shared")
            nc.sync.dma_start(b2_in[:, 0:NL], x2pack[:])
            nc.gpsimd.collective_compute(
                "AllGather", mybir.AluOpType.bypass, replica_groups=rg,
                ins=[b2_in[:]], outs=[b2_out[:]],
            )

            # ---- xmT[r] = (M_r.T @ x2_local) in fp16, computed during AG2.
            xm16 = sb.tile([F, R * NL], F16)
            xm16_v = xm16.rearrange("g (r j) -> g r j", r=R)
            for r in range(R):
                pxm = psh.tile([F, NL], F32, tag="h")
                nc.tensor.matmul(pxm[:], relm_sb[:, r * F:(r + 1) * F],
                                 x2pack[:], start=True, stop=True)
                ceng = nc.vector.tensor_copy if r % 2 == 0 else nc.scalar.copy
                ceng(xm16_v[:, r, :], pxm[:])

            # Load gathered x2 in 8 chunks.
            x2t = sb.tile([F, N], F16)
            for q in range(NCORES):
                eng = nc.sync if q % 2 == 0 else nc.gpsimd
                eng.dma_start(
                    x2t[:, q * NL:(q + 1) * NL],
                    b2_out[q, :, 0:NL],
                )

            # ---- DistMult scores: out[r, n, m] = sum_g xm[r][n, g] x2[m, g]
            # One K=64 fp16 matmul per [128, 512] tile; stage a full 128-row
            # block in SBUF, then store it as ONE contiguous 2 MiB DMA,
            # rotating across the three DGE rings.
            st_engs = [nc.sync, nc.gpsimd, nc.scalar]
            blk = 0
            for r in range(R):
                for nb in range(NB):
                    lhs = xm16_v[:, r, nb * 128:(nb + 1) * 128]
                    so = stage.tile([128, N], F32, tag="so", bufs=3)
                    for mc in range(MC):
                        cs = slice(mc * 512, (mc + 1) * 512)
                        po = pso.tile([128, 512], F32, tag="o")
                        nc.tensor.matmul(po[:], lhs, x2t[:, cs],
                                         start=True, stop=True)
                        if mc % 2 == 0:
                            nc.vector.tensor_copy(so[:, cs], po[:])
                        else:
                            nc.scalar.copy(so[:, cs], po[:])
                    st_engs[blk % 3].dma_start(
                        out[r, nb * 128:(nb + 1) * 128, :], so[:])
                    blk += 1
    nc.compile()
    return nc


def _get_nc():
    global _NC_CACHE
    if _NC_CACHE is None:
        _NC_CACHE = _build()
    return _NC_CACHE


def kernel(**inputs):
    global LAST_RESULT
    A = np.asarray(inputs["adjacency"], dtype=np.float32)
    x0 = np.asarray(inputs["features"], dtype=np.float32)
    W = np.asarray(inputs["conv_weights"], dtype=np.float32)
    Mrel = np.asarray(inputs["rel_matrices"], dtype=np.float32)

    # h1[r, m, g] = sum_f x0[m, f] * W[0, r, g, f]; SBUF layout [p, r, mb, g].
    h1 = np.einsum("mf,rgf->rmg", x0, W[0])
    h1_tiled = np.ascontiguousarray(
        h1.reshape(R, MB, 128, F).transpose(2, 0, 1, 3)
    ).reshape(128, R * MB * F).astype(F8NP)
    # wt2[f, (r, g)] = W[1, r, g, f]
    wt2 = np.ascontiguousarray(
        W[1].transpose(2, 0, 1)).reshape(F, R * F).astype(np.float16)
    # relm[g1, (r, g2)] = M[r, g1, g2]
    relm = np.ascontiguousarray(
        Mrel.transpose(1, 0, 2)).reshape(F, R * F).astype(np.float16)

    nc = _get_nc()
    in_maps = []
    for c in range(NCORES):
        sl = A[:, c * NL:(c + 1) * NL, :]             # [R, NL, N]
        atr = np.ascontiguousarray(
            sl.transpose(0, 2, 1)                      # [R, N(m), NL(j)]
            .reshape(R, MB, 128, NL)
            .transpose(0, 2, 1, 3)                     # [R, p, mb, j]
        ).astype(F8NP)
        in_maps.append(dict(atr=atr, h1=h1_tiled, wt2=wt2, relm=relm))

    res = bass_utils.run_bass_kernel_spmd(
        nc, in_maps, core_ids=list(range(NCORES)), trace=TRACE,
    )
    LAST_RESULT = res

    out = np.empty((R, N, N), dtype=np.float32)
    for c in range(NCORES):
        out[:, c * NL:(c + 1) * NL, :] = res.results[c]["out"]
    return out


# revision 5
# speedup vs baseline: 1.5021x; 1.1022x over previous
"""BasicRGCN Trainium2 kernel (8 NeuronCores, SPMD) - AllReduce formulation.

Math (reference):
    x = features                                   # [N, F]
    for l in 0..1:
        y = sum_r A[r] @ x @ W[l, r].T             # [N, F]
        x = sigmoid(y)
    out[r] = (x @ M_r) @ x.T                       # [R, N, N]

Layer 2 is computed as a PARTIAL SUM per core: core c holds both its
adjacency row-slab A[:, rows_c, :] (for layer 1 + its output slab) and an
adjacency column-slab A[:, :, cols_c]. After layer 1 each core projects only
its LOCAL x1 rows (h2loc = x1_loc @ W2.T, fp8) and computes
y2_partial[g, n] = sum_r sum_{m in cols_c} h2loc_r[m, g] A[r, n, m] for ALL
n. A single fp16 AllReduce of y2_partial then gives every core the full
layer-2 pre-activations - replacing the two AllGathers (x1 and x2) of the
row-parallel formulation and removing the gathered-h2 recompute from the
critical path entirely. Partials are pre-scaled by 1/4 (folded into W2 on
the host) so the fp16 CCE sum cannot overflow; the sigmoid applies scale=4.

Per-core identity (which columns are "mine" for the DistMult xm factor)
enters only through a tiny per-core int32 index input driving one indirect
DMA gather - the NEFF itself is identical on all 8 cores (SPMD).

Precision: fp8 adjacency/h1/h2loc with fp32 PSUM accumulation (layer-2
pre-activations are ~5e4 so sigmoid saturates and absorbs layer error);
fp16 x2/xm DistMult (worst-case rel err ~4e-5 in this regime).

Schedule notes (from ntff profiles of prior revisions):
  * A dummy 2KiB AllGather is triggered as the very first instruction: the
    collectives runtime pays a ~40us init/launch-skew barrier plus ~11us
    first-op setup on the cc stream; this hides it under the load phase so
    the AllReduce starts processing the moment its data is ready.
  * Loads (17 MiB: row-slab, col-slab, h1) rotate across the three DGE
    rings (sync=SP-HWDGE, gpsimd=SWDGE, scalar=ACT-HWDGE); the phase is
    HBM-bound (~358 GB/s) at ~50us, overlapping L1 and the cc barrier.
  * Layer-2 partial accumulation interleaves 4 independent PSUM chains (one
    per 512-col chunk of the active half), consuming column-slab blocks in
    DMA arrival order.
  * Output stores are one fully-contiguous 2 MiB DMA per 128-row block,
    rotated across the three DGE rings (~350 GB/s sustained, HBM-write
    bound - the phase floor).
  * No keep-warm is needed: the PE idles only across the AllReduce, and
    even at the cold-isolated matmul rate the DistMult (one K=64 fp16
    matmul per [128,512] tile) outpaces the stores.
"""

import numpy as np
import ml_dtypes

import concourse.bacc as bacc
import concourse.mybir as mybir
import concourse.tile as tile
from concourse import bass, bass_utils

R, N, F = 4, 4096, 64
NCORES = 8
NL = N // NCORES          # 512 local node rows per core
MB = N // 128             # 32 contraction blocks of 128 (layer 1)
MBL = NL // 128           # 4 local contraction blocks (layer 2 partial)
NB = NL // 128            # 4 output row-blocks per core
MC = N // 512             # 8 column-chunks

WARM0 = 16                # pre-warm matmuls at kernel start
Y2SCALE = 4.0             # fp16 AR partials carry y2/4; sigmoid re-scales

F8NP = ml_dtypes.float8_e4m3fn
F8 = mybir.dt.float8e4
F16 = mybir.dt.float16
F32 = mybir.dt.float32
I32 = mybir.dt.int32

# Set by the test harness to collect a profile; grading path leaves these alone.
TRACE = False
LAST_RESULT = None

_NC_CACHE = None


def _build():
    nc = bacc.Bacc("TRN2", target_bir_lowering=False, debug=False,
                   num_devices=NCORES)

    # Per-core inputs (host pre-laid-out; see kernel() below).
    atr = nc.dram_tensor("atr", [R, 128, MB, NL], F8, kind="ExternalInput")
    atc = nc.dram_tensor("atc", [R, 128, MBL, N], F8, kind="ExternalInput")
    h1 = nc.dram_tensor("h1", [128, R * MB * F], F8, kind="ExternalInput")
    wt2 = nc.dram_tensor("wt2", [F, R * F], F16, kind="ExternalInput")
    relm = nc.dram_tensor("relm", [F, R * F], F16, kind="ExternalInput")
    idx = nc.dram_tensor("idx", [F, 1], I32, kind="ExternalInput")
    out = nc.dram_tensor("out", [R, NL, N], F32, kind="ExternalOutput")

    rg = [list(range(NCORES))]
    SIG = mybir.ActivationFunctionType.Sigmoid

    with tile.TileContext(nc) as tc:
        with (
            tc.tile_pool(name="big", bufs=1) as big,
            tc.tile_pool(name="sb", bufs=1) as sb,
            tc.tile_pool(name="stage", bufs=2) as stage,
            tc.tile_pool(name="psl", bufs=4, space="PSUM") as psl,
            tc.tile_pool(name="psh", bufs=1, space="PSUM") as psh,
            tc.tile_pool(name="pso", bufs=3, space="PSUM") as pso,
            tc.tile_pool(name="dram", bufs=1, space="DRAM") as dram,
        ):
            # ---- CC warmup: trigger a dummy tiny AllGather first thing.
            # Nothing reads its output; it pulls the collectives init /
            # launch-skew barrier into the load window. Input DRAM is
            # uninitialized - AllGather only copies bytes.
            cw_in = dram.tile([F, 16], F16)
            cw_out = dram.tile([NCORES, F, 16], F16, addr_space="Shared")
            nc.gpsimd.collective_compute(
                "AllGather", mybir.AluOpType.bypass, replica_groups=rg,
                ins=[cw_in[:]], outs=[cw_out[:]],
            )

            # Resident adjacency slabs: rows (L1) + columns (L2 partial).
            a_res = big.tile([128, R * MB * NL], F8)
            a_v = a_res.rearrange("p (r mb j) -> p r mb j", r=R, mb=MB)
            a_col = big.tile([128, R * MBL * N], F8)
            ac_v = a_col.rearrange("p (r mb n) -> p r mb n", r=R, mb=MBL)

            h1_sb = sb.tile([128, R * MB * F], F8)
            h1_v = h1_sb.rearrange("p (r mb g) -> p r mb g", r=R, mb=MB)
            wt2_sb = sb.tile([F, R * F], F16)
            relm_sb = sb.tile([F, R * F], F16)
            idx_sb = sb.tile([F, 1], I32)

            rings = [nc.sync, nc.gpsimd, nc.scalar]
            qi = 0

            def ring():
                nonlocal qi
                e = rings[qi % 3]
                qi += 1
                return e

            ring().dma_start(wt2_sb[:], wt2[:])
            ring().dma_start(relm_sb[:], relm[:])
            ring().dma_start(idx_sb[:], idx[:])
            HC = R * MB * F // 4
            for q in range(4):
                ring().dma_start(h1_sb[:, q * HC:(q + 1) * HC],
                                 h1[:, q * HC:(q + 1) * HC])
            # Row-slab first (layer 1 consumes it), then column-slab.
            H = MB // 4
            for r in range(R):
                for h in range(4):
                    ring().dma_start(
                        a_v[:, r, h * H:(h + 1) * H, :],
                        atr[r, :, h * H:(h + 1) * H, :],
                    )
            for r in range(R):
                for mbl in range(MBL):
                    ring().dma_start(
                        ac_v[:, r, mbl, :],
                        atc[r, :, mbl, :],
                    )

            # Pre-warm the PE while the adjacency stream lands. Junk results
            # land in a psl ring buffer; layer 1 resets it with start=True.
            warm = psl.tile([F, NL], F32, tag="acc")
            for _ in range(WARM0):
                nc.tensor.matmul(warm[:], h1_v[:, 0, 0, :],
                                 h1_sb[:, 0:NL], start=True, stop=True)

            # ---- Layer 1: y1T[g, n_local] = sum_{r, m} h1_r[m, g] A[r, n, m]
            x1pack = sb.tile([F, NL], F16)
            y1 = psl.tile([F, NL], F32, tag="acc")
            k = 0
            for r in range(R):
                for mb in range(MB):
                    nc.tensor.matmul(
                        y1[:], h1_v[:, r, mb, :], a_v[:, r, mb, :],
                        start=(k == 0), stop=(k == R * MB - 1),
                    )
                    k += 1
            nc.scalar.activation(x1pack[:], y1[:], SIG)

            # ---- h2loc[m, (r g)] = x1_loc[m, :] @ W2r.T (local rows, fp8)
            h2loc = sb.tile([128, MBL * R * F], F8)
            h2l_v = h2loc.rearrange("p (mb r g) -> p mb r g", mb=MBL, r=R)
            for mbl in range(MBL):
                ph = psh.tile([128, R * F], F32, tag="h")
                nc.tensor.matmul(ph[:], x1pack[:, mbl * 128:(mbl + 1) * 128],
                                 wt2_sb[:], start=True, stop=True)
                ceng = nc.vector.tensor_copy if mbl % 2 == 0 else nc.scalar.copy
                ceng(h2l_v[:, mbl, :, :],
                     ph[:].rearrange("p (r g) -> p r g", r=R))

            # ---- Layer-2 partials: y2p[g, n] = sum_{r, m_loc} h2loc A_col.
            # Two n-halves; 4 interleaved PSUM accumulation chains per half,
            # consuming column-slab blocks in DMA arrival order.
            y2p = sb.tile([F, N], F16)
            for half in range(2):
                acc = [psl.tile([F, 512], F32, tag="acc", name=f"acc{half}_{j}")
                       for j in range(4)]
                k = 0
                for r in range(R):
                    for mbl in range(MBL):
                        for j in range(4):
                            off = half * 2048 + j * 512
                            nc.tensor.matmul(
                                acc[j][:], h2l_v[:, mbl, r, :],
                                ac_v[:, r, mbl, off:off + 512],
                                start=(k == 0), stop=(k == R * MBL - 1),
                            )
                        k += 1
                for j in range(4):
                    off = half * 2048 + j * 512
                    ceng = (nc.vector.tensor_copy if j % 2 == 0
                            else nc.scalar.copy)
                    ceng(y2p[:, off:off + 512], acc[j][:])

            # ---- AllReduce y2 partials (fp16, scaled by 1/4)
            ar_in = dram.tile([F, N], F16)
            ar_out = dram.tile([F, N], F16, addr_space="Shared")
            nc.sync.dma_start(ar_in[:], y2p[:])
            nc.gpsimd.collective_compute(
                "AllReduce", mybir.AluOpType.add, replica_groups=rg,
                ins=[ar_in[:]], outs=[ar_out[:]],
            )

            # ---- Own-columns x2 slice via indirect gather (per-core idx).
            # ar_out viewed as [(g, chunk), 512]; row g*8+c -> partition g.
            y2own = sb.tile([F, NL], F16)
            ar_view = ar_out[:].rearrange("g (c j) -> (g c) j", c=MC)
            nc.gpsimd.indirect_dma_start(
                out=y2own[:], out_offset=None,
                in_=ar_view,
                in_offset=bass.IndirectOffsetOnAxis(ap=idx_sb[:, :1], axis=0),
            )
            x2own = sb.tile([F, NL], F16)
            nc.scalar.activation(x2own[:], y2own[:], SIG, scale=Y2SCALE)

            # ---- Full x2: load AR result, sigmoid per chunk.
            ytmp = sb.tile([F, N], F16)
            x2t = sb.tile([F, N], F16)
            for q in range(MC):
                cs = slice(q * 512, (q + 1) * 512)
                (nc.sync if q % 2 == 0 else nc.scalar).dma_start(
                    ytmp[:, cs], ar_out[:, cs])
                nc.scalar.activation(x2t[:, cs], ytmp[:, cs], SIG,
                                     scale=Y2SCALE)

            # ---- xmT[r] = (M_r.T @ x2_own) in fp16
            xm16 = sb.tile([F, R * NL], F16)
            xm16_v = xm16.rearrange("g (r j) -> g r j", r=R)
            for r in range(R):
                pxm = psl.tile([F, NL], F32, tag="acc")
                nc.tensor.matmul(pxm[:], relm_sb[:, r * F:(r + 1) * F],
                                 x2own[:], start=True, stop=True)
                ceng = nc.vector.tensor_copy if r % 2 == 0 else nc.scalar.copy
                ceng(xm16_v[:, r, :], pxm[:])

            # ---- DistMult scores: out[r, n, m] = sum_g xm[r][n, g] x2[m, g]
            # One K=64 fp16 matmul per [128, 512] tile; stage a full 128-row
            # block in SBUF, store it as ONE contiguous 2 MiB DMA, rotating
            # across the three DGE rings.
            st_engs = [nc.sync, nc.gpsimd, nc.scalar]
            blk = 0
            for r in range(R):
                for nb in range(NB):
                    lhs = xm16_v[:, r, nb * 128:(nb + 1) * 128]
                    so = stage.tile([128, N], F32, tag="so")
                    for mc in range(MC):
                        cs = slice(mc * 512, (mc + 1) * 512)
                        po = pso.tile([128, 512], F32, tag="o")
                        nc.tensor.matmul(po[:], lhs, x2t[:, cs],
                                         start=True, stop=True)
                        if mc % 2 == 0:
                            nc.vector.tensor_copy(so[:, cs], po[:])
                        else:
                            nc.scalar.copy(so[:, cs], po[:])
                    st_engs[blk % 3].dma_start(
                        out[r, nb * 128:(nb + 1) * 128, :], so[:])
                    blk += 1
    nc.compile()
    return nc


def _get_nc():
    global _NC_CACHE
    if _NC_CACHE is None:
        _NC_CACHE = _build()
    return _NC_CACHE


def kernel(**inputs):
    global LAST_RESULT
    A = np.asarray(inputs["adjacency"], dtype=np.float32)
    x0 = np.asarray(inputs["features"], dtype=np.float32)
    W = np.asarray(inputs["conv_weights"], dtype=np.float32)
    Mrel = np.asarray(inputs["rel_matrices"], dtype=np.float32)

    # h1[r, m, g] = sum_f x0[m, f] * W[0, r, g, f]; SBUF layout [p, r, mb, g].
    h1 = np.einsum("mf,rgf->rmg", x0, W[0])
    h1_tiled = np.ascontiguousarray(
        h1.reshape(R, MB, 128, F).transpose(2, 0, 1, 3)
    ).reshape(128, R * MB * F).astype(F8NP)
    # wt2[f, (r, g)] = W[1, r, g, f] / 4 (fp16 AR partial headroom)
    wt2 = np.ascontiguousarray(
        W[1].transpose(2, 0, 1) / Y2SCALE).reshape(F, R * F).astype(np.float16)
    # relm[g1, (r, g2)] = M[r, g1, g2]
    relm = np.ascontiguousarray(
        Mrel.transpose(1, 0, 2)).reshape(F, R * F).astype(np.float16)

    nc = _get_nc()
    in_maps = []
    for c in range(NCORES):
        sl = A[:, c * NL:(c + 1) * NL, :]             # [R, NL, N]
        atr = np.ascontiguousarray(
            sl.transpose(0, 2, 1)                      # [R, N(m), NL(j)]
            .reshape(R, MB, 128, NL)
            .transpose(0, 2, 1, 3)                     # [R, p, mb, j]
        ).astype(F8NP)
        slc = A[:, :, c * NL:(c + 1) * NL]            # [R, N(n), NL(m)]
        atc = np.ascontiguousarray(
            slc.transpose(0, 2, 1)                     # [R, NL(m), N(n)]
            .reshape(R, MBL, 128, N)
            .transpose(0, 2, 1, 3)                     # [R, p, mbl, n]
        ).astype(F8NP)
        idx = (np.arange(F, dtype=np.int32) * MC + c).reshape(F, 1)
        in_maps.append(dict(atr=atr, atc=atc, h1=h1_tiled, wt2=wt2,
                            relm=relm, idx=idx))

    res = bass_utils.run_bass_kernel_spmd(
        nc, in_maps, core_ids=list(range(NCORES)), trace=TRACE,
    )
    LAST_RESULT = res

    out = np.empty((R, N, N), dtype=np.float32)
    for c in range(NCORES):
        out[:, c * NL:(c + 1) * NL, :] = res.results[c]["out"]
    return out
